# revision 18
# baseline (speedup 1.0000x reference)
# Self-contained Trainium2 Bass kernel for nn_AdaAttentionalGNN (B=2, D=256, H=4, N=M=2048, L=6).
# Sharding: data-parallel over batch B across 2 groups of 4 cores; within a group each core
# owns an N/4-column slice of the query axis. Matmuls run as fp32r (12-bit-mantissa fp32 at
# full PE rate); attention probabilities bf16; top-k pruning via on-device bisection with
# exact floor-tie handling mirroring jax.lax.top_k index-order tie-breaking.
import sys
sys.path.insert(0, '/opt/trn_rl_repo')
import numpy as np

import concourse.bass as bass
import concourse.bacc as bacc
import concourse.tile as tile
import concourse.tile_utils as tile_utils
tile_utils.max_sbuf_usage = 208 * 1024
import concourse.mybir as mybir
from concourse.bass_utils import run_bass_kernel_spmd

F32 = mybir.dt.float32
F32R = mybir.dt.float32r
BF16 = mybir.dt.bfloat16
FP16 = mybir.dt.float16
U16 = mybir.dt.uint16
I32 = mybir.dt.int32
ALU = mybir.AluOpType
ACTF = mybir.ActivationFunctionType

D = 256
H = 4
HD = 64
L = 6
NAMES = ("self", "cross", "self", "cross", "self", "cross")
POOLS = (1, 1, 2, 2, 2, 2)
EPS = 1e-5
BISECT = 12
NCORES = 8
GROUPS = [[0, 1, 2, 3], [4, 5, 6, 7]]
PAIRS = ('00', '11', '01', '10')


def head_perm():
    p = np.zeros(D, np.int64)
    for h in range(H):
        for hd in range(HD):
            p[h * HD + hd] = hd * H + h
    return p


def _pack_rows(a):
    C, X = a.shape
    n_hi = C // 128
    return np.ascontiguousarray(
        a.reshape(n_hi, 128, X).transpose(1, 0, 2).reshape(128, n_hi * X))


def _pack_bias(b):
    C = b.shape[0]
    return np.ascontiguousarray(b.reshape(C // 128, 128).T)


def build(NT, NL=L, DBG=False):
    SL = NT // 4
    MT = NT // 128
    NH = SL // 128
    MC = NT // 512          # m-chunks for the prob-acc phase

    nc = bacc.Bacc("TRN2", target_bir_lowering=False, debug=False,
                   enable_asserts=False, num_devices=NCORES)

    d0_d = nc.dram_tensor("d0", [128, 2 * NT], F32R, kind="ExternalInput")
    d1_d = nc.dram_tensor("d1", [128, 2 * NT], F32R, kind="ExternalInput")
    wq_d = nc.dram_tensor("wq", [L, 128, 512], F32R, kind="ExternalInput")
    wk_d = nc.dram_tensor("wk", [L, 128, 512], F32R, kind="ExternalInput")
    wv_d = nc.dram_tensor("wv", [L, 128, 512], F32R, kind="ExternalInput")
    wm_d = nc.dram_tensor("wm", [L, 128, 512], F32R, kind="ExternalInput")
    w1_d = nc.dram_tensor("w1", [L, 128, 2048], F32R, kind="ExternalInput")
    w2_d = nc.dram_tensor("w2", [L, 128, 1024], F32R, kind="ExternalInput")
    bias_d = nc.dram_tensor("bias", [L, 128, 16], F32, kind="ExternalInput")
    bvr_d = nc.dram_tensor("bvr", [L, 1, 256], F32R, kind="ExternalInput")
    out_d = nc.dram_tensor("out", [128, 4 * SL], F32, kind="ExternalOutput")

    desc = [nc.alloc_sbuf_tensor("desc0", [128, 2 * NT], F32R),
            nc.alloc_sbuf_tensor("desc1", [128, 2 * NT], F32R)]
    fl_sb = {p: nc.alloc_sbuf_tensor(f"fl_{p}", [128, NH], FP16) for p in PAIRS}
    ones_bf = nc.alloc_sbuf_tensor("ones_bf", [128, 512], BF16)
    ones128r = nc.alloc_sbuf_tensor("ones128r", [1, 128], F32R)

    prob_dram = {}
    for p in PAIRS:
        for par in range(2):
            prob_dram[(p, par)] = nc.dram_tensor(f"prob_{p}_{par}", [128, NH * NT], FP16)
    eT_dram = nc.dram_tensor("eT_dram", [128, H * NH * NT], BF16)
    ag_in = [nc.dram_tensor(f"ag_in{i}", [128, 4 * SL], F32) for i in range(2)]
    ag_out = [nc.dram_tensor(f"ag_out{i}", [4 * 128, 4 * SL], F32) for i in range(2)]
    ar_in = [nc.dram_tensor(f"ar_in{i}", [128, 8], F32) for i in range(4)]
    ar_out = [nc.dram_tensor(f"ar_out{i}", [128, 8], F32) for i in range(4)]
    rz_dram = nc.dram_tensor("rz_bounce", [1, SL], F32)

    kcnt = {p: NT for p in PAIRS}
    had_mask = {p: False for p in PAIRS}

    with tile.TileContext(nc) as tc:
        from contextlib import ExitStack
        ctx = ExitStack()
        wpool = ctx.enter_context(tc.tile_pool(name="wpool", bufs=1))
        kpool = ctx.enter_context(tc.tile_pool(name="kpool", bufs=1))
        vpool = ctx.enter_context(tc.tile_pool(name="vpool", bufs=1))
        qpool = ctx.enter_context(tc.tile_pool(name="qpool", bufs=1))
        epool = ctx.enter_context(tc.tile_pool(name="epool", bufs=2))
        apool = ctx.enter_context(tc.tile_pool(name="apool", bufs=1))
        a2pool = ctx.enter_context(tc.tile_pool(name="a2pool", bufs=2))
        mpool = ctx.enter_context(tc.tile_pool(name="mpool", bufs=1))
        spool = ctx.enter_context(tc.tile_pool(name="spool", bufs=2))
        zpool = ctx.enter_context(tc.tile_pool(name="zpool", bufs=1))
        z2pool = ctx.enter_context(tc.tile_pool(name="z2pool", bufs=2))
        gpool = ctx.enter_context(tc.tile_pool(name="gpool", bufs=1))
        psA = ctx.enter_context(tc.tile_pool(name="psA", bufs=2, space="PSUM"))
        psB = ctx.enter_context(tc.tile_pool(name="psB", bufs=2, space="PSUM"))
        psC = ctx.enter_context(tc.tile_pool(name="psC", bufs=2, space="PSUM"))

        nc.vector.memset(ones_bf[:, :], 1.0)
        onesf = spool.tile([1, 128], F32, tag="c128")
        nc.vector.memset(onesf[:], 1.0)
        nc.vector.tensor_copy(ones128r[:, :], onesf[:])

        nc.sync.dma_start(desc[0][:, :], d0_d[:, :])
        nc.sync.dma_start(desc[1][:, :], d1_d[:, :])

        pid = nc.vector.partition_id()
        off = (pid % 4) * SL

        def load_weights(l):
            w = {}
            for nm, dram, width in (("wq", wq_d, 512), ("wk", wk_d, 512),
                                    ("wv", wv_d, 512), ("wm", wm_d, 512),
                                    ("w1", w1_d, 2048), ("w2", w2_d, 1024)):
                t = wpool.tile([128, width], F32R, tag=nm)
                nc.sync.dma_start(t[:], dram[l, :, :])
                w[nm] = t
            bt = wpool.tile([128, 16], F32, tag="bias")
            nc.sync.dma_start(bt[:], bias_d[l, :, :])
            w["bias"] = bt
            bv = wpool.tile([1, 256], F32R, tag="bvr")
            nc.sync.dma_start(bv[:], bvr_d[l, :, :])
            w["bvr"] = bv
            return w

        def build_mask(pair, par, k, with_ties):
            acc = apool.tile([128, NH * NT], FP16, tag="maccin")
            nc.sync.dma_start(acc[:], prob_dram[(pair, par)][:, :])
            lo = spool.tile([128, NH], F32, tag="lo")
            hi = spool.tile([128, NH], F32, tag="hi")
            cntlo = spool.tile([128, NH], F32, tag="cntlo")
            mid = spool.tile([128, NH], F32, tag="mid")
            cm = spool.tile([128, NH], F32, tag="cm")
            ge = spool.tile([128, NH], I32, tag="ge")
            gei = spool.tile([128, NH], I32, tag="gei")
            flf = spool.tile([128, NH], F32, tag="flf")
            scratch = apool.tile([128, NT], FP16, tag="mscr")
            kf = float(k)
            nc.vector.memset(hi[:], 64.0 / k)
            if with_ties:
                nc.vector.tensor_copy(flf[:], fl_sb[pair][:, :])
                nc.vector.tensor_copy(lo[:], flf[:])
                for nh in range(NH):
                    nc.vector.tensor_scalar(scratch[:], acc[:, nh * NT:(nh + 1) * NT],
                                            lo[:, nh:nh + 1], 0.0, ALU.is_gt, ALU.add,
                                            accum_out=cntlo[:, nh:nh + 1])
            else:
                nc.vector.memset(lo[:], 0.0)
                nc.vector.memset(cntlo[:], float(NT))
            for _ in range(BISECT):
                nc.vector.tensor_add(mid[:], lo[:], hi[:])
                nc.vector.tensor_scalar_mul(mid[:], mid[:], 0.5)
                for nh in range(NH):
                    nc.vector.tensor_scalar(scratch[:], acc[:, nh * NT:(nh + 1) * NT],
                                            mid[:, nh:nh + 1], 0.0, ALU.is_gt, ALU.add,
                                            accum_out=cm[:, nh:nh + 1])
                nc.vector.tensor_scalar(ge[:], cm[:], kf, None, ALU.is_ge)
                nc.vector.tensor_scalar(gei[:], cm[:], kf, None, ALU.is_lt)
                nc.vector.copy_predicated(lo[:], ge[:], mid[:])
                nc.vector.copy_predicated(cntlo[:], ge[:], cm[:])
                nc.vector.copy_predicated(hi[:], gei[:], mid[:])
            maskT = mpool.tile([128, MT * SL], U16, tag="maskT")
            mview = maskT[:, :].rearrange("p (mh n) -> p mh n", mh=MT)
            if with_ties:
                r = spool.tile([128, NH], F32, tag="rfill")
                nc.vector.tensor_scalar(r[:], cntlo[:], -1.0, kf, ALU.mult, ALU.add)
                tiet = apool.tile([128, NT], FP16, tag="mtie")
                cumt = apool.tile([128, NT], FP16, tag="mcum")
                maint = apool.tile([128, NT], FP16, tag="mscr")
                for nh in range(NH):
                    a_nh = acc[:, nh * NT:(nh + 1) * NT]
                    minv = a2pool.tile([128, NT], U16, tag="minv")
                    nc.vector.tensor_scalar(maint[:], a_nh, lo[:, nh:nh + 1], None, ALU.is_gt)
                    nc.vector.tensor_scalar(tiet[:], a_nh, flf[:, nh:nh + 1], None, ALU.is_equal)
                    nc.vector.tensor_tensor_scan(cumt[:], tiet[:], tiet[:], 0.0,
                                                 ALU.add, ALU.bypass)
                    nc.vector.scalar_tensor_tensor(cumt[:], cumt[:], r[:, nh:nh + 1],
                                                   tiet[:], ALU.is_le, ALU.mult)
                    nc.vector.tensor_tensor(maint[:], maint[:], cumt[:], ALU.add)
                    nc.vector.tensor_scalar(minv[:], maint[:], 0.5, None, ALU.is_lt)
                    nc.scalar.dma_start_transpose(mview[:, :, nh * 128:(nh + 1) * 128],
                                                  minv[:])
            else:
                for nh in range(NH):
                    minv = a2pool.tile([128, NT], U16, tag="minv")
                    nc.vector.tensor_scalar(minv[:], acc[:, nh * NT:(nh + 1) * NT],
                                            lo[:, nh:nh + 1], None, ALU.is_le)
                    nc.scalar.dma_start_transpose(mview[:, :, nh * 128:(nh + 1) * 128],
                                                  minv[:])
            return maskT

        def attn_prop(l, w, pair, ti, si, maskT, feeds_prune, last):
            dt_, ds_ = desc[ti], desc[si]
            bias = w["bias"]
            etd = eT_dram[:, :].rearrange("p (h nh m) -> p h nh m", h=H, nh=NH)
            # xsl: this core's query-column slice of the target desc
            xsl = qpool.tile([128, 2 * SL], F32R, tag="xsl")
            for ch in range(2):
                nc.vector.tensor_copy(xsl[:, ch * SL:(ch + 1) * SL],
                                      dt_[:, bass.ds(off + ch * NT, SL)])
            # q
            q = qpool.tile([128, 2 * SL], F32R, tag="q")
            for mt_ in range(2):
                qp = psC.tile([128, 512], F32, tag="psc")
                for kt in range(2):
                    nc.tensor.matmul(qp[:, 0:SL],
                                     w["wq"][:, kt * 256 + mt_ * 128: kt * 256 + mt_ * 128 + 128],
                                     xsl[:, kt * SL:(kt + 1) * SL],
                                     start=(kt == 0), stop=(kt == 1))
                nc.scalar.activation(q[:, mt_ * SL:(mt_ + 1) * SL], qp[:, 0:SL],
                                     ACTF.Identity, bias=bias[:, 0 + mt_:1 + mt_])
            # k (full source)
            ksb = kpool.tile([128, 2 * NT], F32R, tag="k")
            for mt_ in range(2):
                for nt in range(NT // 512):
                    kp = psA.tile([128, max(512, 2 * SL)], F32, tag="sps")
                    for kt in range(2):
                        nc.tensor.matmul(kp[:, 0:512],
                                         w["wk"][:, kt * 256 + mt_ * 128: kt * 256 + mt_ * 128 + 128],
                                         ds_[:, kt * NT + nt * 512: kt * NT + (nt + 1) * 512],
                                         start=(kt == 0), stop=(kt == 1))
                    nc.scalar.activation(ksb[:, mt_ * NT + nt * 512: mt_ * NT + (nt + 1) * 512],
                                         kp[:, 0:512], ACTF.Identity,
                                         bias=bias[:, 2 + mt_:3 + mt_])
            # vT with interleaved ones columns (bias bv added via ones-row matmul)
            vT = vpool.tile([128, MT * 260], BF16, tag="vT")
            vview = vT[:, :].rearrange("p (mh c) -> p mh c", mh=MT)
            nc.vector.memset(vview[:, :, 64::65], 1.0)
            for mt_ in range(MT):
                vp = psC.tile([128, 512], F32, tag="psc")
                for kt in range(2):
                    nc.tensor.matmul(vp[:, 0:256],
                                     ds_[:, kt * NT + mt_ * 128: kt * NT + mt_ * 128 + 128],
                                     w["wv"][:, kt * 256:(kt + 1) * 256],
                                     start=(kt == 0), stop=False)
                nc.tensor.matmul(vp[:, 0:256], ones128r[:, :], w["bvr"][:, :],
                                 start=False, stop=True)
                for h in range(H):
                    nc.vector.tensor_copy(vview[:, mt_, h * 65: h * 65 + 64],
                                          vp[:, h * 64:(h + 1) * 64])
            # per-head attention; e' streamed in 2-m-tile chunks
            attall = zpool.tile([128, 2 * SL], F32R, tag="attall")
            r4s = []
            flt = flt32 = None
            if feeds_prune:
                flt = spool.tile([128, NH], FP16, tag="flt")
                flt32 = spool.tile([128, NH], F32, tag="flt32")
            for h in range(H):
                po, chh = (h % 2) * 64, h // 2
                ap_ = psB.tile([65, SL], F32, tag="attps")
                for mt2 in range(MT // 2):
                    sp = psA.tile([128, max(512, 2 * SL)], F32, tag="sps")
                    for half in range(2):
                        mt_ = mt2 * 2 + half
                        nc.tensor.matmul(sp[:, half * SL:(half + 1) * SL],
                                         ksb[po:po + 64, chh * NT + mt_ * 128: chh * NT + mt_ * 128 + 128],
                                         q[po:po + 64, chh * SL: chh * SL + SL],
                                         start=True, stop=True)
                    esb = epool.tile([128, 2 * SL], BF16, tag="eh")
                    nc.scalar.activation(esb[:, 0:2 * SL], sp[:, 0:2 * SL], ACTF.Exp)
                    for half in range(2):
                        mt_ = mt2 * 2 + half
                        esl = esb[:, half * SL:(half + 1) * SL]
                        if maskT is not None:
                            nc.vector.copy_predicated(esl, maskT[:, mt_ * SL:(mt_ + 1) * SL],
                                                      ones_bf[:, 0:SL])
                        nc.tensor.matmul(ap_[:],
                                         vT[:, mt_ * 260 + h * 65: mt_ * 260 + h * 65 + 65],
                                         esl, start=(mt_ == 0), stop=(mt_ == MT - 1))
                        if feeds_prune:
                            etc = epool.tile([128, NH * 128], BF16, tag="etc")
                            ecv = etc[:, :].rearrange("p (nh m) -> p nh m", nh=NH)
                            nc.scalar.dma_start_transpose(ecv[:, :, :], esl)
                            nc.sync.dma_start(etd[:, h, :, mt_ * 128:(mt_ + 1) * 128],
                                              ecv[:, :, :])
                # Z -> [128, NH] via DRAM bounce, fast partition-parallel reciprocal
                zrow = gpool.tile([1, SL], F32, tag="zrow")
                nc.vector.tensor_copy(zrow[0:1, 0:SL], ap_[64:65, 0:SL])
                nc.sync.dma_start(rz_dram[0:1, 0:SL], zrow[0:1, 0:SL])
                r4z = spool.tile([128, NH], F32, tag="r4z")
                nc.sync.dma_start(
                    r4z[:, 0:NH],
                    rz_dram[0:1, 0:SL].rearrange("o (nh p) -> (o p) nh", p=128))
                r4r = spool.tile([128, NH], F32, tag=f"r4h{h}")
                nc.vector.reciprocal(r4r[:], r4z[:])
                nc.sync.dma_start(
                    rz_dram[0:1, 0:SL].rearrange("o (nh p) -> (o p) nh", p=128),
                    r4r[:, 0:NH])
                rrowr = gpool.tile([1, SL], F32R, tag="rrowr")
                nc.gpsimd.dma_start(rrowr[0:1, 0:SL], rz_dram[0:1, 0:SL])
                rb = psC.tile([128, 512], F32, tag="psc")
                nc.tensor.matmul(rb[0:64, 0:SL], ones128r[0:1, 0:64], rrowr[0:1, 0:SL],
                                 start=True, stop=True)
                rbs = gpool.tile([64, SL], F32, tag="rbs")
                nc.vector.tensor_copy(rbs[:], rb[0:64, 0:SL])
                nc.vector.tensor_tensor(attall[po:po + 64, chh * SL: chh * SL + SL],
                                        ap_[0:64, 0:SL], rbs[:], ALU.mult)
                if feeds_prune:
                    r4 = spool.tile([128, NH], F32, tag=f"r4s{h}")
                    nc.vector.tensor_scalar_mul(r4[:], r4r[:], 16.0)
                    r4s.append(r4)
                    for nh in range(NH):
                        if h == 0:
                            nc.vector.tensor_scalar(flt32[:, nh:nh + 1],
                                                    ones_bf[:, 0:1],
                                                    r4[:, nh:nh + 1], None, ALU.mult)
                        else:
                            fout = flt if h == H - 1 else flt32
                            nc.vector.scalar_tensor_tensor(fout[:, nh:nh + 1],
                                                           ones_bf[:, 0:1],
                                                           r4[:, nh:nh + 1],
                                                           flt32[:, nh:nh + 1],
                                                           ALU.mult, ALU.add)
            # prob-acc phase: chunked over m, reading eT back from DRAM
            if feeds_prune:
                par_wr = (l // 2) % 2
                pview = prob_dram[(pair, par_wr)][:, :].rearrange(
                    "p (nh m) -> p nh m", nh=NH)
                for mc in range(MC):
                    acch = apool.tile([128, NH * 512], F32, tag="acch")
                    a16 = apool.tile([128, NH * 512], FP16, tag="a16")
                    for h in range(H):
                        for hf in range(2):
                            etr = a2pool.tile([128, NH * 256], BF16, tag="etr")
                            nc.sync.dma_start(
                                etr[:, :].rearrange("p (nh m) -> p nh m", nh=NH),
                                etd[:, h, :, mc * 512 + hf * 256: mc * 512 + (hf + 1) * 256])
                            for nh in range(NH):
                                aslc = slice(nh * 512 + hf * 256, nh * 512 + (hf + 1) * 256)
                                eslc = slice(nh * 256, (nh + 1) * 256)
                                if h == 0:
                                    nc.vector.tensor_scalar(acch[:, aslc], etr[:, eslc],
                                                            r4s[h][:, nh:nh + 1], None, ALU.mult)
                                else:
                                    aout = a16 if h == H - 1 else acch
                                    nc.vector.scalar_tensor_tensor(aout[:, aslc], etr[:, eslc],
                                                                   r4s[h][:, nh:nh + 1],
                                                                   acch[:, aslc],
                                                                   ALU.mult, ALU.add)
                    nc.sync.dma_start(pview[:, :, mc * 512:(mc + 1) * 512],
                                      a16[:, :].rearrange("p (nh m) -> p nh m", nh=NH))
            # merge
            msg = zpool.tile([128, 2 * SL], F32R, tag="msg")
            for mt_ in range(2):
                mp = psA.tile([128, max(512, 2 * SL)], F32, tag="sps")
                for kt in range(2):
                    nc.tensor.matmul(mp[:, 0:SL],
                                     w["wm"][:, kt * 256 + mt_ * 128: kt * 256 + mt_ * 128 + 128],
                                     attall[:, kt * SL:(kt + 1) * SL],
                                     start=(kt == 0), stop=(kt == 1))
                nc.scalar.activation(msg[:, mt_ * SL:(mt_ + 1) * SL], mp[:, 0:SL],
                                     ACTF.Identity, bias=bias[:, 6 + mt_:7 + mt_])
            # W1 + instnorm stats
            zsb = zpool.tile([128, 4 * SL], F32, tag="zsb")
            zsum = spool.tile([128, 4], F32, tag="zsum")
            zss = spool.tile([128, 4], F32, tag="zss")
            sqs = z2pool.tile([128, 2 * SL], F32, tag="delta")
            for mt_ in range(4):
                zp = psA.tile([128, max(512, 2 * SL)], F32, tag="sps")
                for kt in range(4):
                    rhs = xsl[:, kt * SL:(kt + 1) * SL] if kt < 2 else \
                          msg[:, (kt - 2) * SL:(kt - 1) * SL]
                    nc.tensor.matmul(zp[:, 0:SL],
                                     w["w1"][:, kt * 512 + mt_ * 128: kt * 512 + mt_ * 128 + 128],
                                     rhs, start=(kt == 0), stop=(kt == 3))
                nc.scalar.activation(zsb[:, mt_ * SL:(mt_ + 1) * SL], zp[:, 0:SL],
                                     ACTF.Identity, bias=bias[:, 8 + mt_:9 + mt_],
                                     accum_out=zsum[:, mt_:mt_ + 1])
                nc.scalar.activation(sqs[:, 0:SL], zsb[:, mt_ * SL:(mt_ + 1) * SL],
                                     ACTF.Square, accum_out=zss[:, mt_:mt_ + 1])
            sidx = (l % 2) * 2 + ti
            stpack = spool.tile([128, 8], F32, tag="stpack")
            nc.vector.tensor_copy(stpack[:, 0:4], zsum[:])
            nc.vector.tensor_copy(stpack[:, 4:8], zss[:])
            nc.sync.dma_start(ar_in[sidx][:, :], stpack[:])
            nc.gpsimd.collective_compute("AllReduce", ALU.add, replica_groups=GROUPS,
                                         ins=[ar_in[sidx][:, :].opt()],
                                         outs=[ar_out[sidx][:, :].opt()])
            stat = spool.tile([128, 8], F32, tag="stat")
            nc.sync.dma_start(stat[:], ar_out[sidx][:, :])
            return dict(zsb=zsb, stat=stat, flt=flt)

        def attn_prop_tail(l, w, c, ti, last):
            bias = w["bias"]
            zsb, stat = c["zsb"], c["stat"]
            mu = spool.tile([128, 4], F32, tag="mu")
            var = spool.tile([128, 4], F32, tag="var")
            sd = spool.tile([128, 4], F32, tag="sd")
            rstd = spool.tile([128, 4], F32, tag="rstd")
            nbias = spool.tile([128, 4], F32, tag="nbias")
            tmp = spool.tile([128, 4], F32, tag="tmp4")
            inv_n = 1.0 / NT
            nc.vector.tensor_scalar_mul(mu[:], stat[:, 0:4], inv_n)
            nc.vector.tensor_scalar_mul(var[:], stat[:, 4:8], inv_n)
            nc.vector.tensor_tensor(tmp[:], mu[:], mu[:], ALU.mult)
            nc.vector.tensor_tensor(var[:], var[:], tmp[:], ALU.subtract)
            epst = spool.tile([128, 1], F32, tag="epst")
            nc.vector.memset(epst[:], EPS)
            nc.scalar.activation(sd[:], var[:], ACTF.Sqrt, bias=epst[:, 0:1])
            nc.vector.reciprocal(rstd[:], sd[:])
            nc.vector.tensor_tensor(nbias[:], mu[:], rstd[:], ALU.mult)
            nc.vector.tensor_scalar_mul(nbias[:], nbias[:], -1.0)
            hsb = zpool.tile([128, 4 * SL], F32R, tag="hsb")
            for mt_ in range(4):
                nc.scalar.activation(hsb[:, mt_ * SL:(mt_ + 1) * SL],
                                     zsb[:, mt_ * SL:(mt_ + 1) * SL],
                                     ACTF.Relu, bias=nbias[:, mt_:mt_ + 1],
                                     scale=rstd[:, mt_:mt_ + 1])
            delta = z2pool.tile([128, 2 * SL], F32, tag="delta")
            for mt_ in range(2):
                dp = psA.tile([128, max(512, 2 * SL)], F32, tag="sps")
                for kt in range(4):
                    nc.tensor.matmul(dp[:, 0:SL],
                                     w["w2"][:, kt * 256 + mt_ * 128: kt * 256 + mt_ * 128 + 128],
                                     hsb[:, kt * SL:(kt + 1) * SL],
                                     start=(kt == 0), stop=(kt == 3))
                nc.scalar.activation(delta[:, mt_ * SL:(mt_ + 1) * SL], dp[:, 0:SL],
                                     ACTF.Identity, bias=bias[:, 14 + mt_:15 + mt_])
            if last:
                for ch in range(2):
                    nc.vector.tensor_tensor(delta[:, ch * SL:(ch + 1) * SL],
                                            delta[:, ch * SL:(ch + 1) * SL],
                                            desc[ti][:, bass.ds(off + ch * NT, SL)],
                                            ALU.add)
                    nc.sync.dma_start(out_d[:, (ti * 2 + ch) * SL:(ti * 2 + ch + 1) * SL],
                                      delta[:, ch * SL:(ch + 1) * SL])
            else:
                for ch in range(2):
                    nc.sync.dma_start(ag_in[l % 2][:, (ti * 2 + ch) * SL:(ti * 2 + ch + 1) * SL],
                                      delta[:, ch * SL:(ch + 1) * SL])

        # ===== layers =====
        for l in range(NL):
            w = load_weights(l)
            pairs = [('01', 0, 1), ('10', 1, 0)] if NAMES[l] == 'cross' else \
                    [('00', 0, 0), ('11', 1, 1)]
            masked = POOLS[l] != 1
            feeds = l <= 3
            par_rd = ((l - 2) // 2) % 2
            masks = {}
            if masked:
                for (pair, ti, si) in pairs:
                    k_new = kcnt[pair] // POOLS[l]
                    masks[pair] = build_mask(pair, par_rd, k_new, had_mask[pair])
                    kcnt[pair] = k_new
            ctxs = {}
            for (pair, ti, si) in pairs:
                ctxs[pair] = attn_prop(l, w, pair, ti, si, masks.get(pair),
                                       feeds, l == NL - 1)
                if feeds:
                    nc.vector.tensor_copy(fl_sb[pair][:, :], ctxs[pair]["flt"][:])
                    if masked:
                        had_mask[pair] = True
            for (pair, ti, si) in pairs:
                attn_prop_tail(l, w, ctxs[pair], ti, l == NL - 1)
            if l < NL - 1:
                nc.gpsimd.collective_compute("AllGather", ALU.bypass, replica_groups=GROUPS,
                                             ins=[ag_in[l % 2][:, :].opt()],
                                             outs=[ag_out[l % 2][:, :].opt()])
                for s in range(4):
                    for d_i in range(2):
                        agu = gpool.tile([128, 2 * SL], F32, tag="agu")
                        nc.sync.dma_start(agu[:, :],
                                          ag_out[l % 2][s * 128:(s + 1) * 128,
                                                        d_i * 2 * SL:(d_i + 1) * 2 * SL])
                        for ch in range(2):
                            nc.vector.tensor_tensor(
                                desc[d_i][:, ch * NT + s * SL: ch * NT + (s + 1) * SL],
                                desc[d_i][:, ch * NT + s * SL: ch * NT + (s + 1) * SL],
                                agu[:, ch * SL:(ch + 1) * SL],
                                ALU.add)
        ctx.close()

    nc.compile()
    return nc


def prep_inputs(inputs, NT):
    perm = head_perm()
    iq = np.float32(1.0 / np.sqrt(HD))
    f32 = lambda a: np.asarray(a, np.float32)
    wq = np.stack([_pack_rows((f32(inputs['Wq'][l])[perm] * iq).T) for l in range(L)])
    wk = np.stack([_pack_rows(f32(inputs['Wk'][l])[perm].T) for l in range(L)])
    wv = np.stack([_pack_rows(f32(inputs['Wv'][l])[perm].T) for l in range(L)])
    wm = np.stack([_pack_rows(f32(inputs['Wm'][l]).T[perm]) for l in range(L)])
    w1 = np.stack([_pack_rows(f32(inputs['W1'][l]).T) for l in range(L)])
    w2 = np.stack([_pack_rows(f32(inputs['W2'][l]).T) for l in range(L)])
    bias = np.zeros((L, 128, 16), np.float32)
    bvr = np.zeros((L, 1, 256), np.float32)
    for l in range(L):
        bias[l, :, 0:2] = _pack_bias(f32(inputs['bq'][l])[perm] * iq)
        bias[l, :, 2:4] = _pack_bias(f32(inputs['bk'][l])[perm])
        bias[l, :, 6:8] = _pack_bias(f32(inputs['bm'][l]))
        bias[l, :, 8:12] = _pack_bias(f32(inputs['b1'][l]))
        bias[l, :, 14:16] = _pack_bias(f32(inputs['b2'][l]))
        bvr[l, 0, :] = f32(inputs['bv'][l])[perm]
    in_maps = []
    for c in range(NCORES):
        b = c // 4
        in_maps.append({
            "d0": _pack_rows(f32(inputs['desc0'][b])),
            "d1": _pack_rows(f32(inputs['desc1'][b])),
            "wq": wq, "wk": wk, "wv": wv, "wm": wm, "w1": w1, "w2": w2,
            "bias": bias, "bvr": bvr,
        })
    return in_maps


def assemble_out(results, NT):
    SL = NT // 4
    out = np.zeros((2, 2, D, NT), np.float32)
    for c in range(NCORES):
        b, s = c // 4, c % 4
        blob = results[c]["out"].reshape(128, 2, 2, SL)
        for d_i in range(2):
            for ch in range(2):
                out[d_i, b, ch * 128:(ch + 1) * 128, s * SL:(s + 1) * SL] = blob[:, d_i, ch]
    return out


_NC_CACHE = {}


def get_nc(NT, NL=L, DBG=False):
    key = (NT, NL, DBG)
    if key not in _NC_CACHE:
        _NC_CACHE[key] = build(NT, NL, DBG)
    return _NC_CACHE[key]


def run(inputs, NT, NL=L, DBG=False, **kw):
    nc = get_nc(NT, NL, DBG)
    in_maps = prep_inputs(inputs, NT)
    res = run_bass_kernel_spmd(nc, in_maps, core_ids=list(range(NCORES)), **kw)
    return assemble_out(res.results, NT), res


def kernel(**inputs):
    NT = int(np.asarray(inputs['desc0']).shape[2])
    out, _ = run(inputs, NT)
    return out
